# revision 8
# baseline (speedup 1.0000x reference)
"""Trainium2 Bass kernel for nn_MemModule (topk_masking).

Computes, for x[B,N,C], mem_weight[M,C], v_w[C,C], v_b[C]:
    fn = l2norm(x.reshape(T,C)); wn = l2norm(mem_weight)
    alpha = softmax(fn @ wn.T); thr = 20th largest per row
    alpha = l2norm(where(alpha >= thr, alpha, 0))
    vmem = mem_weight @ v_w.T + v_b
    y = (alpha @ vmem).reshape(B,N,C)
    att = alpha.reshape(B,N,M).transpose(0,2,1)
    loss = mean((x-y)^2) + 2e-4 * mean(-sum(att*log(att+1e-12), axis=1))

Key identity used: the softmax denominator and the row-max subtraction both
cancel under the final L2 renormalization, so the kernel works directly with
e = exp(logit + ln(rinv)) and only the top-20 mask and rinv matter.

Sharding: data-parallel over the flattened token axis; with B=8 and 8 cores,
core b handles batch b exactly. mem_weight / v_w / v_b are replicated.
"""

import numpy as np

B, N, C, M = 8, 2048, 1024, 2000
TOPK = 20
ENT_W = 2.0e-4
NCORES = 8
P = 128                    # partitions
NT = N // P                # token tiles per core (16)
NMC = (M + P - 1) // P     # memory-slot chunks (16, last=80)
NCC = C // P               # channel chunks (8)
LB = 500                   # logits column block
NLB = M // LB              # 4
ST = 2                     # token tiles staged per att flush
GROUPS = 16                # top-k stage-1 groups
GW = M // GROUPS           # 125

_BUILT = None


def _m_chunks():
    out = []
    for i in range(NMC):
        m0 = i * P
        out.append((i, m0, min(P, M - m0)))
    return out


def _build():
    import concourse.bacc as bacc
    import concourse.mybir as mybir
    import concourse.tile as tile
    from concourse import bass as bass_mod

    dt = mybir.dt
    Alu = mybir.AluOpType
    Act = mybir.ActivationFunctionType

    nc = bacc.Bacc("TRN2", target_bir_lowering=False, debug=False,
                   enable_asserts=False)

    x_d = nc.dram_tensor("x", [N, C], dt.float32, kind="ExternalInput").ap()
    mem_d = nc.dram_tensor("mem_weight", [M, C], dt.float32, kind="ExternalInput").ap()
    vw_d = nc.dram_tensor("v_w", [C, C], dt.float32, kind="ExternalInput").ap()
    vb_d = nc.dram_tensor("v_b", [C], dt.float32, kind="ExternalInput").ap()
    eye16_d = nc.dram_tensor("eye16", [P, P], dt.float16, kind="ExternalInput").ap()
    eye32_d = nc.dram_tensor("eye32", [P, P], dt.float32, kind="ExternalInput").ap()

    y_d = nc.dram_tensor("y", [N, C], dt.float32, kind="ExternalOutput").ap()
    att_d = nc.dram_tensor("att", [M, N], dt.float32, kind="ExternalOutput").ap()
    par_d = nc.dram_tensor("partials", [P, 2 * NT], dt.float32, kind="ExternalOutput").ap()

    with tile.TileContext(nc) as tc:
        from contextlib import ExitStack
        with ExitStack() as ctx:
            const = ctx.enter_context(tc.tile_pool(name="const", bufs=1))
            persist = ctx.enter_context(tc.tile_pool(name="persist", bufs=1))

            eye16 = const.tile([P, P], dt.float16)
            nc.sync.dma_start(out=eye16, in_=eye16_d)
            eye32f = const.tile([P, P], dt.float32)
            nc.sync.dma_start(out=eye32f, in_=eye32_d)
            eye32 = const.tile([P, P], dt.float32r)
            nc.vector.tensor_copy(eye32, eye32f)
            eps12 = const.tile([P, 1], dt.float32)
            nc.vector.memset(eps12, 1.0e-12)
            vb_bc = const.tile([P, C], dt.float32)
            vb_bcast_ap = bass_mod.AP(
                tensor=vb_d.tensor, offset=vb_d.offset,
                ap=[[0, P]] + list(vb_d.ap))
            nc.sync.dma_start(out=vb_bc, in_=vb_bcast_ap)

            wnT_h = persist.tile([P, NCC, M], dt.float16)
            wnT_l = persist.tile([P, NCC, M], dt.float16)
            vmem = persist.tile([P, NMC, C], dt.float32r)
            norms = persist.tile([P, NMC], dt.float32)
            ssd_acc = persist.tile([P, 2 * NT], dt.float32)

            # ---------------- setup: wn normalization + transposes ----------
            with ExitStack() as sctx:
                sp = sctx.enter_context(tc.tile_pool(name="setup", bufs=3))
                sps = sctx.enter_context(tc.tile_pool(name="setup_small", bufs=4))
                pss = sctx.enter_context(
                    tc.tile_pool(name="setup_psum", bufs=2, space="PSUM"))

                for (i, m0, rows) in _m_chunks():
                    mem_s = sp.tile([P, C], dt.float32, tag="mem_s")
                    nc.sync.dma_start(out=mem_s[:rows], in_=mem_d[m0:m0 + rows, :])
                    junk = sp.tile([P, C], dt.float32, tag="junk")
                    ssm = sps.tile([P, 1], dt.float32, tag="ssm")
                    nc.vector.scalar_tensor_tensor(
                        out=junk[:rows], in0=mem_s[:rows], scalar=1.0,
                        in1=mem_s[:rows], op0=Alu.mult, op1=Alu.mult,
                        accum_out=ssm[:rows])
                    nc.scalar.sqrt(norms[:rows, i:i + 1], ssm[:rows])
                    invm = sps.tile([P, 1], dt.float32, tag="invm")
                    nc.vector.reciprocal(invm[:rows], norms[:rows, i:i + 1])
                    wh = sp.tile([P, C], dt.float16, tag="wh")
                    nc.scalar.activation(wh[:rows], mem_s[:rows], Act.Copy,
                                         scale=invm[:rows])
                    wl = sp.tile([P, C], dt.float16, tag="wl")
                    nc.vector.scalar_tensor_tensor(
                        out=wl[:rows], in0=mem_s[:rows], scalar=invm[:rows],
                        in1=wh[:rows], op0=Alu.mult, op1=Alu.subtract)
                    for (src, dst) in ((wh, wnT_h), (wl, wnT_l)):
                        for w in range(2):
                            pw = pss.tile([P, 4 * P], dt.float16, tag="pw")
                            for k in range(4):
                                cb = w * 4 + k
                                nc.tensor.transpose(
                                    pw[:, k * P:k * P + rows],
                                    src[:rows, cb * P:(cb + 1) * P],
                                    eye16[:rows, :rows])
                            nc.scalar.activation(
                                dst[:, w * 4:(w + 1) * 4, m0:m0 + rows],
                                pw.rearrange("p (k t) -> p k t", k=4)[:, :, 0:rows],
                                Act.Copy)

                # v_w -> v_wT (fp16)
                vwT = sctx.enter_context(tc.tile_pool(name="vwT", bufs=1))
                v_wT = vwT.tile([P, NCC, C], dt.float16)
                for j in range(NCC):
                    vws = sp.tile([P, C], dt.float32, tag="mem_s")
                    nc.sync.dma_start(out=vws, in_=vw_d[j * P:(j + 1) * P, :])
                    vwh = sp.tile([P, C], dt.float16, tag="wh")
                    nc.scalar.activation(vwh, vws, Act.Copy)
                    for w in range(2):
                        pw = pss.tile([P, 4 * P], dt.float16, tag="pw")
                        for k in range(4):
                            cb = w * 4 + k
                            nc.tensor.transpose(
                                pw[:, k * P:(k + 1) * P],
                                vwh[:, cb * P:(cb + 1) * P], eye16)
                        nc.scalar.activation(
                            v_wT[:, w * 4:(w + 1) * 4, j * P:(j + 1) * P],
                            pw.rearrange("p (k t) -> p k t", k=4), Act.Copy)

                # vmem[m, c] = norms[m] * (wnT_h.T @ v_wT) + v_b
                for (i, m0, rows) in _m_chunks():
                    for h in range(2):
                        pv = pss.tile([P, C // 2], dt.float32, tag="pv")
                        for cch in range(NCC):
                            nc.tensor.matmul(
                                pv[:rows],
                                wnT_h[:, cch, m0:m0 + rows],
                                v_wT[:, cch, h * (C // 2):(h + 1) * (C // 2)],
                                start=(cch == 0), stop=(cch == NCC - 1))
                        nc.vector.scalar_tensor_tensor(
                            out=vmem[:rows, i, h * (C // 2):(h + 1) * (C // 2)],
                            in0=pv[:rows], scalar=norms[:rows, i:i + 1],
                            in1=vb_bc[:rows, h * (C // 2):(h + 1) * (C // 2)],
                            op0=Alu.mult, op1=Alu.add)

            # ---------------- main loop over token tiles ---------------------
            mp = ctx.enter_context(tc.tile_pool(name="main", bufs=2))
            mp1 = ctx.enter_context(tc.tile_pool(name="main1", bufs=1))
            msm = ctx.enter_context(tc.tile_pool(name="msmall", bufs=2))
            pf = ctx.enter_context(tc.tile_pool(name="pf", bufs=2, space="PSUM"))
            pL = ctx.enter_context(tc.tile_pool(name="pL", bufs=2, space="PSUM"))
            pa = ctx.enter_context(tc.tile_pool(name="pa", bufs=2, space="PSUM"))
            py = ctx.enter_context(tc.tile_pool(name="py", bufs=2, space="PSUM"))

            att3 = att_d[0:NMC * P - P, :].rearrange("(ch p) n -> p ch n", p=P)

            for j in range(NT):
                t0 = j * P
                x_s = mp.tile([P, C], dt.float32, tag="x")
                nc.sync.dma_start(out=x_s, in_=x_d[t0:t0 + P, :])

                scr = mp1.tile([P, C], dt.float32, tag="scr")
                ss = msm.tile([P, 1], dt.float32, tag="ss")
                nc.vector.scalar_tensor_tensor(
                    out=scr, in0=x_s, scalar=1.0, in1=x_s,
                    op0=Alu.mult, op1=Alu.mult, accum_out=ss)
                nrm = msm.tile([P, 1], dt.float32, tag="nrm")
                nc.scalar.sqrt(nrm, ss)
                invn = msm.tile([P, 1], dt.float32, tag="invn")
                nc.vector.reciprocal(invn, nrm)

                fn_h = mp.tile([P, C], dt.float16, tag="fn_h")
                nc.scalar.activation(fn_h, x_s, Act.Copy, scale=invn)
                fn_l = mp.tile([P, C], dt.float16, tag="fn_l")
                nc.vector.scalar_tensor_tensor(
                    out=fn_l, in0=x_s, scalar=invn, in1=fn_h,
                    op0=Alu.mult, op1=Alu.subtract)

                fnT_h = mp.tile([P, NCC, P], dt.float16, tag="fnT_h")
                fnT_l = mp.tile([P, NCC, P], dt.float16, tag="fnT_l")
                for (src, dst, eng) in ((fn_h, fnT_h, "act"), (fn_l, fnT_l, "dve")):
                    for w in range(2):
                        pw = pf.tile([P, 4 * P], dt.float16, tag="pw")
                        for k in range(4):
                            cb = w * 4 + k
                            nc.tensor.transpose(
                                pw[:, k * P:(k + 1) * P],
                                src[:, cb * P:(cb + 1) * P], eye16)
                        dstap = dst[:, w * 4:(w + 1) * 4, :]
                        pwap = pw.rearrange("p (k t) -> p k t", k=4)
                        if eng == "act":
                            nc.scalar.activation(dstap, pwap, Act.Copy)
                        else:
                            nc.vector.tensor_copy(dstap, pwap)

                # logits: 3-pass fp16 split, one psum group per 500-col block
                Lfull = mp.tile([P, M], dt.float32, tag="Lfull")
                for b in range(NLB):
                    c0 = b * LB
                    pb = pL.tile([P, LB], dt.float32, tag="pb")
                    first = True
                    for (fa, wb) in ((fnT_h, wnT_h), (fnT_h, wnT_l), (fnT_l, wnT_h)):
                        for cch in range(NCC):
                            nc.tensor.matmul(
                                pb, fa[:, cch, :], wb[:, cch, c0:c0 + LB],
                                start=first,
                                stop=(not first and cch == NCC - 1 and fa is fnT_l))
                            first = False
                    if b % 2 == 0:
                        nc.scalar.activation(Lfull[:, c0:c0 + LB], pb, Act.Copy)
                    else:
                        nc.vector.tensor_copy(Lfull[:, c0:c0 + LB], pb)

                # top-k: grouped max -> 128 candidates -> top-24
                cand = msm.tile([P, GROUPS * 8], dt.float32, tag="cand")
                for g in range(GROUPS):
                    nc.vector.max(out=cand[:, g * 8:(g + 1) * 8],
                                  in_=Lfull[:, g * GW:(g + 1) * GW])
                t24 = msm.tile([P, 24], dt.float32, tag="t24")
                cscr = msm.tile([P, GROUPS * 8], dt.float32, tag="cscr")
                nc.vector.max(out=t24[:, 0:8], in_=cand)
                nc.vector.match_replace(out=cscr, in_to_replace=t24[:, 0:8],
                                        in_values=cand, imm_value=-2.0)
                nc.vector.max(out=t24[:, 8:16], in_=cscr)
                nc.vector.match_replace(out=cscr, in_to_replace=t24[:, 8:16],
                                        in_values=cscr, imm_value=-2.0)
                nc.vector.max(out=t24[:, 16:24], in_=cscr)
                thr = t24[:, TOPK - 1:TOPK]

                # rinv = 1/||exp(top20)||; bias = ln(rinv) folded into exp
                e24 = msm.tile([P, 24], dt.float32, tag="e24")
                nc.scalar.activation(e24[:, 0:TOPK], t24[:, 0:TOPK], Act.Exp)
                j24 = msm.tile([P, 24], dt.float32, tag="j24")
                ssq = msm.tile([P, 1], dt.float32, tag="ssq")
                nc.vector.scalar_tensor_tensor(
                    out=j24[:, 0:TOPK], in0=e24[:, 0:TOPK], scalar=1.0,
                    in1=e24[:, 0:TOPK], op0=Alu.mult, op1=Alu.mult,
                    accum_out=ssq)
                nrm20 = msm.tile([P, 1], dt.float32, tag="nrm20")
                nc.scalar.sqrt(nrm20, ssq)
                rinv = msm.tile([P, 1], dt.float32, tag="rinv")
                nc.vector.reciprocal(rinv, nrm20)
                lnr = msm.tile([P, 1], dt.float32, tag="lnr")
                nc.scalar.activation(lnr, rinv, Act.Ln)

                e_s = mp1.tile([P, M], dt.float32r, tag="e_s")
                nc.scalar.activation(e_s, Lfull, Act.Exp, bias=lnr)

                # masked (float32r) written in place over e_s
                e_r = e_s
                nc.vector.scalar_tensor_tensor(
                    out=e_r, in0=Lfull, scalar=thr, in1=e_s,
                    op0=Alu.is_ge, op1=Alu.mult)

                # entropy partials: sum a*ln(a) over top 20
                a20 = msm.tile([P, 24], dt.float32, tag="a20")
                nc.vector.tensor_scalar_mul(a20[:, 0:TOPK], e24[:, 0:TOPK], rinv)
                lna = msm.tile([P, 24], dt.float32, tag="lna")
                nc.scalar.activation(lna[:, 0:TOPK], a20[:, 0:TOPK], Act.Ln,
                                     bias=eps12)
                nc.vector.scalar_tensor_tensor(
                    out=j24[:, 0:TOPK], in0=a20[:, 0:TOPK], scalar=1.0,
                    in1=lna[:, 0:TOPK], op0=Alu.mult, op1=Alu.mult,
                    accum_out=ssd_acc[:, NT + j:NT + j + 1])

                # transpose masked alpha -> staging (float32r)
                if j % ST == 0:
                    stage = mp1.tile([P, NMC, ST * P], dt.float32r, tag="stage")
                jo = (j % ST) * P
                MP = M - 15 * P  # 80, the partial last m-block
                for w in range(4):
                    pwv = pa.tile([P, 4 * P], dt.float32r, tag="pwv")
                    for k in range(4):
                        mb = w * 4 + k
                        mc0 = mb * P
                        rows = min(P, M - mc0)
                        nc.tensor.transpose(
                            pwv[0:rows, k * P:(k + 1) * P],
                            e_r[:, mc0:mc0 + rows], eye32)
                    pwap = pwv.rearrange("p (k t) -> p k t", k=4)
                    if w < 3:
                        if w % 2 == 0:
                            nc.scalar.activation(
                                stage[:, w * 4:(w + 1) * 4, jo:jo + P],
                                pwap, Act.Copy)
                        else:
                            nc.vector.tensor_copy(
                                stage[:, w * 4:(w + 1) * 4, jo:jo + P], pwap)
                    else:
                        nc.vector.tensor_copy(
                            stage[:, 12:15, jo:jo + P], pwap[:, 0:3, :])
                        nc.scalar.activation(
                            stage[0:MP, 15, jo:jo + P],
                            pwv[0:MP, 3 * P:4 * P], Act.Copy)

                # y = alphaT.T @ vmem  (float32r)
                ysb = mp1.tile([P, C], dt.float32, tag="ysb")
                for h in range(2):
                    pyv = py.tile([P, C // 2], dt.float32, tag="pyv")
                    for (i, m0, rows) in _m_chunks():
                        nc.tensor.matmul(
                            pyv, stage[0:rows, i, jo:jo + P],
                            vmem[0:rows, i, h * (C // 2):(h + 1) * (C // 2)],
                            start=(i == 0), stop=(i == NMC - 1))
                    if h == 0:
                        nc.scalar.activation(ysb[:, 0:C // 2], pyv, Act.Copy)
                    else:
                        nc.vector.tensor_copy(ysb[:, C // 2:C], pyv)
                nc.sync.dma_start(out=y_d[t0:t0 + P, :], in_=ysb)

                # loss partials: sum (x - y)^2
                nc.vector.scalar_tensor_tensor(
                    out=scr, in0=ysb, scalar=-1.0, in1=x_s,
                    op0=Alu.mult, op1=Alu.add)
                nc.scalar.activation(scr, scr, Act.Square,
                                     accum_out=ssd_acc[:, j:j + 1])

                # att store every ST tiles
                if j % ST == ST - 1:
                    n0 = (j - (ST - 1)) * P
                    stf = stage.bitcast(dt.float32)
                    nc.sync.dma_start(
                        out=att3[:, :, n0:n0 + ST * P],
                        in_=stf[:, 0:NMC - 1, :])
                    nc.sync.dma_start(
                        out=att_d[15 * P:M, n0:n0 + ST * P],
                        in_=stf[0:M - 15 * P, NMC - 1, :])

            nc.sync.dma_start(out=par_d, in_=ssd_acc)

    nc.compile()
    return nc


def _get_built():
    global _BUILT
    if _BUILT is None:
        _BUILT = _build()
    return _BUILT


def kernel(x, mem_weight, v_w, v_b):
    from concourse import bass_utils

    nc = _get_built()
    eye16 = np.eye(P, dtype=np.float16)
    eye32 = np.eye(P, dtype=np.float32)
    in_maps = []
    for b in range(NCORES):
        in_maps.append({
            "x": np.ascontiguousarray(x[b], dtype=np.float32),
            "mem_weight": np.ascontiguousarray(mem_weight, dtype=np.float32),
            "v_w": np.ascontiguousarray(v_w, dtype=np.float32),
            "v_b": np.ascontiguousarray(v_b, dtype=np.float32),
            "eye16": eye16,
            "eye32": eye32,
        })
    res = bass_utils.run_bass_kernel_spmd(nc, in_maps, core_ids=list(range(NCORES)))

    y = np.stack([res.results[b]["y"] for b in range(NCORES)])
    att = np.stack([res.results[b]["att"] for b in range(NCORES)])
    ssd = 0.0
    ent = 0.0
    for b in range(NCORES):
        par = res.results[b]["partials"].astype(np.float64)
        ssd += par[:, 0:NT].sum()
        ent += par[:, NT:2 * NT].sum()
    loss = ssd / (B * N * C) + ENT_W * (-ent / (B * N))
    return y, att, np.float32(loss)


# revision 11
# speedup vs baseline: 1.4376x; 1.4376x over previous
"""Trainium2 Bass kernel for nn_MemModule (topk_masking).

Per core (= one batch b of 8): cosine-sim scores of 2048 tokens against
2000 memory slots, top-20 mask + L2 renorm, reconstruction y = alpha@vmem,
transposed attention output, and loss partials.

Math identities used:
  - softmax denominator and row-max shift cancel under L2 renorm, so only
    e = exp(logit + ln(rinv)) with rinv = 1/||exp(top20 logits)|| is needed;
    ln(rinv) = -0.5*ln(sum(exp(2*l_i))) folds into the exp bias.
  - entropy: ln(alpha_i) = l_i + ln(rinv) exactly, so the entropy partial
    needs no extra log pass.
  - logits at fp32 quality via a 3-pass fp16 hi/lo split matmul (PE fp16
    preserves subnormals; verified 0 top-20 flips vs fp64 on the data).

Pipeline: one-tile software skew — PE computes tile j's logits while the
DVE/ACT top-k -> exp -> mask chain of tile j-1 runs, keeping the PE dense
(HAM stays at full clock).
"""

import numpy as np

B, N, C, M = 8, 2048, 1024, 2000
TOPK = 20
ENT_W = 2.0e-4
NCORES = 8
P = 128                    # partitions
NT = N // P                # token tiles per core (16)
NMC = (M + P - 1) // P     # memory-slot chunks (16, last=80)
NCC = C // P               # channel chunks (8)
LB = 500                   # logits column block
NLB = M // LB              # 4
GROUPS = 16                # top-k stage-1 groups
GW = M // GROUPS           # 125
MP = M - 15 * P            # 80, partial last m-block

_BUILT = None


def _m_chunks():
    return [(i, i * P, min(P, M - i * P)) for i in range(NMC)]


def _build():
    import concourse.bacc as bacc
    import concourse.mybir as mybir
    import concourse.tile as tile
    from concourse import bass as bass_mod

    dt = mybir.dt
    Alu = mybir.AluOpType
    Act = mybir.ActivationFunctionType

    nc = bacc.Bacc("TRN2", target_bir_lowering=False, debug=False,
                   enable_asserts=False)

    x_d = nc.dram_tensor("x", [N, C], dt.float32, kind="ExternalInput").ap()
    mem_d = nc.dram_tensor("mem_weight", [M, C], dt.float32, kind="ExternalInput").ap()
    vw_d = nc.dram_tensor("v_w", [C, C], dt.float32, kind="ExternalInput").ap()
    vb_d = nc.dram_tensor("v_b", [C], dt.float32, kind="ExternalInput").ap()
    eye16_d = nc.dram_tensor("eye16", [P, P], dt.float16, kind="ExternalInput").ap()
    eye32_d = nc.dram_tensor("eye32", [P, P], dt.float32, kind="ExternalInput").ap()

    y_d = nc.dram_tensor("y", [N, C], dt.float32, kind="ExternalOutput").ap()
    att_d = nc.dram_tensor("att", [M, N], dt.float32, kind="ExternalOutput").ap()
    par_d = nc.dram_tensor("partials", [P, 2 * NT], dt.float32, kind="ExternalOutput").ap()

    with tile.TileContext(nc) as tc:
        from contextlib import ExitStack
        with ExitStack() as ctx:
            const = ctx.enter_context(tc.tile_pool(name="const", bufs=1))
            persist = ctx.enter_context(tc.tile_pool(name="persist", bufs=1))

            eye16 = const.tile([P, P], dt.float16)
            nc.sync.dma_start(out=eye16, in_=eye16_d)
            eye32 = const.tile([P, P], dt.float32r)

            wnT_h = persist.tile([P, NCC, M], dt.float16)
            wnT_l = persist.tile([P, NCC, M], dt.float16)
            vmem = persist.tile([P, NMC, C], dt.float32r)
            norms = persist.tile([P, NMC], dt.float32)
            ssd_acc = persist.tile([P, 2 * NT], dt.float32)

            # ---------------- setup: wn normalization + transposes ----------
            with ExitStack() as sctx:
                sp = sctx.enter_context(tc.tile_pool(name="setup", bufs=3))
                sps = sctx.enter_context(tc.tile_pool(name="setup_small", bufs=4))
                pss = sctx.enter_context(
                    tc.tile_pool(name="setup_psum", bufs=2, space="PSUM"))

                eye32f = sp.tile([P, P], dt.float32, tag="eye32f")
                nc.sync.dma_start(out=eye32f, in_=eye32_d)
                nc.vector.tensor_copy(eye32, eye32f)
                vb_bc = sp.tile([P, C], dt.float32, tag="vb_bc")
                vb_bcast_ap = bass_mod.AP(
                    tensor=vb_d.tensor, offset=vb_d.offset,
                    ap=[[0, P]] + list(vb_d.ap))
                nc.sync.dma_start(out=vb_bc, in_=vb_bcast_ap)

                for (i, m0, rows) in _m_chunks():
                    mem_s = sp.tile([P, C], dt.float32, tag="mem_s")
                    nc.sync.dma_start(out=mem_s[:rows], in_=mem_d[m0:m0 + rows, :])
                    junk = sp.tile([P, C], dt.float32, tag="junk")
                    ssm = sps.tile([P, 1], dt.float32, tag="ssm")
                    nc.vector.scalar_tensor_tensor(
                        out=junk[:rows], in0=mem_s[:rows], scalar=1.0,
                        in1=mem_s[:rows], op0=Alu.mult, op1=Alu.mult,
                        accum_out=ssm[:rows])
                    nc.scalar.sqrt(norms[:rows, i:i + 1], ssm[:rows])
                    invm = sps.tile([P, 1], dt.float32, tag="invm")
                    nc.vector.reciprocal(invm[:rows], norms[:rows, i:i + 1])
                    wh = sp.tile([P, C], dt.float16, tag="wh")
                    nc.scalar.activation(wh[:rows], mem_s[:rows], Act.Copy,
                                         scale=invm[:rows])
                    wl = sp.tile([P, C], dt.float16, tag="wl")
                    nc.vector.scalar_tensor_tensor(
                        out=wl[:rows], in0=mem_s[:rows], scalar=invm[:rows],
                        in1=wh[:rows], op0=Alu.mult, op1=Alu.subtract)
                    for (src, dst) in ((wh, wnT_h), (wl, wnT_l)):
                        for w in range(2):
                            pw = pss.tile([P, 4 * P], dt.float16, tag="pw")
                            for k in range(4):
                                cb = w * 4 + k
                                nc.tensor.transpose(
                                    pw[:, k * P:k * P + rows],
                                    src[:rows, cb * P:(cb + 1) * P],
                                    eye16[:rows, :rows])
                            nc.scalar.activation(
                                dst[:, w * 4:(w + 1) * 4, m0:m0 + rows],
                                pw.rearrange("p (k t) -> p k t", k=4)[:, :, 0:rows],
                                Act.Copy)

                # v_w -> v_wT (fp16)
                vwT = sctx.enter_context(tc.tile_pool(name="vwT", bufs=1))
                v_wT = vwT.tile([P, NCC, C], dt.float16)
                for j in range(NCC):
                    vws = sp.tile([P, C], dt.float32, tag="mem_s")
                    nc.sync.dma_start(out=vws, in_=vw_d[j * P:(j + 1) * P, :])
                    vwh = sp.tile([P, C], dt.float16, tag="wh")
                    nc.scalar.activation(vwh, vws, Act.Copy)
                    for w in range(2):
                        pw = pss.tile([P, 4 * P], dt.float16, tag="pw")
                        for k in range(4):
                            cb = w * 4 + k
                            nc.tensor.transpose(
                                pw[:, k * P:(k + 1) * P],
                                vwh[:, cb * P:(cb + 1) * P], eye16)
                        nc.scalar.activation(
                            v_wT[:, w * 4:(w + 1) * 4, j * P:(j + 1) * P],
                            pw.rearrange("p (k t) -> p k t", k=4), Act.Copy)

                # vmem[m, c] = norms[m] * (wnT_h.T @ v_wT) + v_b
                for (i, m0, rows) in _m_chunks():
                    for h in range(2):
                        pv = pss.tile([P, C // 2], dt.float32, tag="pv")
                        for cch in range(NCC):
                            nc.tensor.matmul(
                                pv[:rows],
                                wnT_h[:, cch, m0:m0 + rows],
                                v_wT[:, cch, h * (C // 2):(h + 1) * (C // 2)],
                                start=(cch == 0), stop=(cch == NCC - 1))
                        nc.vector.scalar_tensor_tensor(
                            out=vmem[:rows, i, h * (C // 2):(h + 1) * (C // 2)],
                            in0=pv[:rows], scalar=norms[:rows, i:i + 1],
                            in1=vb_bc[:rows, h * (C // 2):(h + 1) * (C // 2)],
                            op0=Alu.mult, op1=Alu.add)

            # ---------------- main loop: 1-tile software skew ----------------
            mp3 = ctx.enter_context(tc.tile_pool(name="main3", bufs=3))
            mp2 = ctx.enter_context(tc.tile_pool(name="main2", bufs=2))
            mp1 = ctx.enter_context(tc.tile_pool(name="main1", bufs=1))
            msm = ctx.enter_context(tc.tile_pool(name="msmall", bufs=2))
            pf = ctx.enter_context(tc.tile_pool(name="pf", bufs=2, space="PSUM"))
            pL = ctx.enter_context(tc.tile_pool(name="pL", bufs=2, space="PSUM"))
            pa = ctx.enter_context(tc.tile_pool(name="pa", bufs=2, space="PSUM"))
            py = ctx.enter_context(tc.tile_pool(name="py", bufs=2, space="PSUM"))

            att3 = att_d[0:15 * P, :].rearrange("(ch p) n -> p ch n", p=P)

            def phase_a(j):
                """DMA x, normalize, fp16 hi/lo, transposes, 3-pass logits."""
                t0 = j * P
                x_s = mp3.tile([P, C], dt.float32, tag="x")
                nc.sync.dma_start(out=x_s, in_=x_d[t0:t0 + P, :])

                junkc = mp2.tile([P, C], dt.float32, tag="junkc")
                ss = msm.tile([P, 1], dt.float32, tag="ss")
                nc.vector.scalar_tensor_tensor(
                    out=junkc, in0=x_s, scalar=1.0, in1=x_s,
                    op0=Alu.mult, op1=Alu.mult, accum_out=ss)
                nrm = msm.tile([P, 1], dt.float32, tag="nrm")
                nc.scalar.sqrt(nrm, ss)
                invn = msm.tile([P, 1], dt.float32, tag="invn")
                nc.vector.reciprocal(invn, nrm)

                fn_h = mp2.tile([P, C], dt.float16, tag="fn_h")
                nc.scalar.activation(fn_h, x_s, Act.Copy, scale=invn)
                fn_l = mp2.tile([P, C], dt.float16, tag="fn_l")
                nc.vector.scalar_tensor_tensor(
                    out=fn_l, in0=x_s, scalar=invn, in1=fn_h,
                    op0=Alu.mult, op1=Alu.subtract)

                fnT_h = mp1.tile([P, NCC, P], dt.float16, tag="fnT_h")
                fnT_l = mp1.tile([P, NCC, P], dt.float16, tag="fnT_l")
                for (src, dst, eng) in ((fn_h, fnT_h, "act"), (fn_l, fnT_l, "dve")):
                    for w in range(2):
                        pw = pf.tile([P, 4 * P], dt.float16, tag="pw")
                        for k in range(4):
                            cb = w * 4 + k
                            nc.tensor.transpose(
                                pw[:, k * P:(k + 1) * P],
                                src[:, cb * P:(cb + 1) * P], eye16)
                        dstap = dst[:, w * 4:(w + 1) * 4, :]
                        pwap = pw.rearrange("p (k t) -> p k t", k=4)
                        if eng == "act":
                            nc.scalar.activation(dstap, pwap, Act.Copy)
                        else:
                            nc.vector.tensor_copy(dstap, pwap)

                Lfull = mp3.tile([P, M], dt.float32, tag="Lfull")
                for b in range(NLB):
                    c0 = b * LB
                    pb = pL.tile([P, LB], dt.float32, tag="pb")
                    first = True
                    for (fa, wb) in ((fnT_h, wnT_h), (fnT_h, wnT_l), (fnT_l, wnT_h)):
                        for cch in range(NCC):
                            nc.tensor.matmul(
                                pb, fa[:, cch, :], wb[:, cch, c0:c0 + LB],
                                start=first,
                                stop=(not first and cch == NCC - 1 and fa is fnT_l))
                            first = False
                    if b % 2 == 0:
                        nc.scalar.activation(Lfull[:, c0:c0 + LB], pb, Act.Copy)
                    else:
                        nc.vector.tensor_copy(Lfull[:, c0:c0 + LB], pb)
                return x_s, Lfull

            def phase_b(j, x_s, Lfull):
                """top-k chain, exp, mask, transposes, y, stores for tile j."""
                cand = msm.tile([P, GROUPS * 8], dt.float32, tag="cand")
                for g in range(GROUPS):
                    nc.vector.max(out=cand[:, g * 8:(g + 1) * 8],
                                  in_=Lfull[:, g * GW:(g + 1) * GW])
                t24 = msm.tile([P, 24], dt.float32, tag="t24")
                cscr = msm.tile([P, GROUPS * 8], dt.float32, tag="cscr")
                nc.vector.max(out=t24[:, 0:8], in_=cand)
                nc.vector.match_replace(out=cscr, in_to_replace=t24[:, 0:8],
                                        in_values=cand, imm_value=-2.0)
                nc.vector.max(out=t24[:, 8:16], in_=cscr)
                nc.vector.match_replace(out=cscr, in_to_replace=t24[:, 8:16],
                                        in_values=cscr, imm_value=-2.0)
                nc.vector.max(out=t24[:, 16:24], in_=cscr)
                thr = t24[:, TOPK - 1:TOPK]

                # ssq = sum exp(2*l_i) over top20; lnr = -0.5*ln(ssq)
                j24 = msm.tile([P, 24], dt.float32, tag="j24")
                ssq = msm.tile([P, 1], dt.float32, tag="ssq")
                nc.scalar.activation(j24[:, 0:TOPK], t24[:, 0:TOPK], Act.Exp,
                                     scale=2.0, accum_out=ssq)
                lnr0 = msm.tile([P, 1], dt.float32, tag="lnr0")
                nc.scalar.activation(lnr0, ssq, Act.Ln)
                lnr = msm.tile([P, 1], dt.float32, tag="lnr")
                nc.vector.tensor_scalar_mul(lnr, lnr0, -0.5)

                e_s = mp1.tile([P, M], dt.float32r, tag="e_s")
                nc.scalar.activation(e_s, Lfull, Act.Exp, bias=lnr)
                nc.vector.scalar_tensor_tensor(
                    out=e_s, in0=Lfull, scalar=thr, in1=e_s,
                    op0=Alu.is_ge, op1=Alu.mult)

                # entropy partial: sum a*(l + lnr), a = exp(l + lnr)
                a20 = msm.tile([P, 24], dt.float32, tag="a20")
                nc.scalar.activation(a20[:, 0:TOPK], t24[:, 0:TOPK], Act.Exp,
                                     bias=lnr)
                nc.vector.scalar_tensor_tensor(
                    out=j24[:, 0:TOPK], in0=t24[:, 0:TOPK], scalar=lnr,
                    in1=a20[:, 0:TOPK], op0=Alu.add, op1=Alu.mult,
                    accum_out=ssd_acc[:, NT + j:NT + j + 1])

                # transpose masked alpha -> staging (float32r)
                stage = mp1.tile([P, NMC, P], dt.float32r, tag="stage")
                for w in range(4):
                    pwv = pa.tile([P, 4 * P], dt.float32r, tag="pwv")
                    for k in range(4):
                        mb = w * 4 + k
                        mc0 = mb * P
                        rows = min(P, M - mc0)
                        nc.tensor.transpose(
                            pwv[0:rows, k * P:(k + 1) * P],
                            e_s[:, mc0:mc0 + rows], eye32)
                    pwap = pwv.rearrange("p (k t) -> p k t", k=4)
                    if w < 3:
                        if w % 2 == 0:
                            nc.scalar.activation(
                                stage[:, w * 4:(w + 1) * 4, :], pwap, Act.Copy)
                        else:
                            nc.vector.tensor_copy(
                                stage[:, w * 4:(w + 1) * 4, :], pwap)
                    else:
                        nc.vector.tensor_copy(
                            stage[:, 12:15, :], pwap[:, 0:3, :])
                        nc.scalar.activation(
                            stage[0:MP, 15, :], pwv[0:MP, 3 * P:4 * P], Act.Copy)

                # y = alphaT.T @ vmem  (float32r)
                ysb = mp1.tile([P, C], dt.float32, tag="ysb")
                for h in range(2):
                    pyv = py.tile([P, C // 2], dt.float32, tag="pyv")
                    for (i, m0, rows) in _m_chunks():
                        nc.tensor.matmul(
                            pyv, stage[0:rows, i, :],
                            vmem[0:rows, i, h * (C // 2):(h + 1) * (C // 2)],
                            start=(i == 0), stop=(i == NMC - 1))
                    if h == 0:
                        nc.scalar.activation(ysb[:, 0:C // 2], pyv, Act.Copy)
                    else:
                        nc.vector.tensor_copy(ysb[:, C // 2:C], pyv)
                t0 = j * P
                nc.gpsimd.dma_start(out=y_d[t0:t0 + P, :], in_=ysb)

                # loss partial: sum (x - y)^2
                nc.gpsimd.tensor_sub(x_s, x_s, ysb)
                junkd = mp2.tile([P, C], dt.float32, tag="junkc")
                nc.vector.scalar_tensor_tensor(
                    out=junkd, in0=x_s, scalar=1.0, in1=x_s,
                    op0=Alu.mult, op1=Alu.mult,
                    accum_out=ssd_acc[:, j:j + 1])

                # att store
                stf = stage.bitcast(dt.float32)
                nc.gpsimd.dma_start(
                    out=att3[:, :, t0:t0 + P], in_=stf[:, 0:15, :])
                nc.gpsimd.dma_start(
                    out=att_d[15 * P:M, t0:t0 + P], in_=stf[0:MP, 15, :])

            prev = phase_a(0)
            for j in range(1, NT):
                cur = phase_a(j)
                phase_b(j - 1, *prev)
                prev = cur
            phase_b(NT - 1, *prev)

            nc.sync.dma_start(out=par_d, in_=ssd_acc)

    nc.compile()
    return nc


def _get_built():
    global _BUILT
    if _BUILT is None:
        _BUILT = _build()
    return _BUILT


def kernel(x, mem_weight, v_w, v_b):
    from concourse import bass_utils

    nc = _get_built()
    eye16 = np.eye(P, dtype=np.float16)
    eye32 = np.eye(P, dtype=np.float32)
    in_maps = []
    for b in range(NCORES):
        in_maps.append({
            "x": np.ascontiguousarray(x[b], dtype=np.float32),
            "mem_weight": np.ascontiguousarray(mem_weight, dtype=np.float32),
            "v_w": np.ascontiguousarray(v_w, dtype=np.float32),
            "v_b": np.ascontiguousarray(v_b, dtype=np.float32),
            "eye16": eye16,
            "eye32": eye32,
        })
    res = bass_utils.run_bass_kernel_spmd(nc, in_maps, core_ids=list(range(NCORES)))

    y = np.stack([res.results[b]["y"] for b in range(NCORES)])
    att = np.stack([res.results[b]["att"] for b in range(NCORES)])
    ssd = 0.0
    ent = 0.0
    for b in range(NCORES):
        par = res.results[b]["partials"].astype(np.float64)
        ssd += par[:, 0:NT].sum()
        ent += par[:, NT:2 * NT].sum()
    loss = ssd / (B * N * C) + ENT_W * (-ent / (B * N))
    return y, att, np.float32(loss)
